# revision 31
# baseline (speedup 1.0000x reference)
"""Trainium2 Bass kernel for a 2-layer GNN (dense message passing) + MLP readout.

Reference computation (N=16384 nodes, D=64 features, G=128 graphs):
    adj_hat = adj + I
    x  = adj_hat @ x_in                 # prop 1
    x  = relu(x @ W1.T + b1)            # fc1
    x  = adj_hat @ x                    # prop 2
    x  = relu(x @ W2.T + b2)            # fc2
    out = segment_sum(x, idx, 128)      # readout
    out = relu(out @ W3.T + b3)
    out = out @ W4.T + b4
    return log_softmax(out, axis=1)

Sharding over 8 NeuronCores: row-shard adj_hat over output nodes (2048 rows
per core), pre-transposed on the host with self-loops folded in so the
contraction index j lands on SBUF partitions.

Performance design (per core, DMA-roofline bound):
 - adjacency/x/h1 in fp8e4 + MatmulPerfMode.DoubleRow: halves HBM bytes vs
   fp16 and runs the PE at full 128-wide rate (the 64-wide feature dim
   would otherwise idle half the array).
 - the adjacency shard streams as 32 mega-chunks of [512 j, 2048 i]
   (1 MiB each, 8 KiB per partition per DMA); 19 of them stay resident in
   SBUF after prop 1 so prop 2 re-reads only 13 from HBM.
 - DMA instruction count is kept minimal (~55/rep): every dma_start
   serializes ~0.65us on the global HWDGE descriptor generator, so x and
   the gathered h1 are loaded with one bulk DMA each into a shared weight
   tile, and h1 is written back with a single DMA.
 - h1 is scaled by 1/16 (folded into W1, compensated in W2/W3) so its fp8
   cast and the fp16 hp tiles stay in range.
"""

import os
import sys
import time

for _p in ("/opt/trn_rl_repo",):
    if _p not in sys.path and os.path.isdir(_p):
        sys.path.insert(0, _p)

import numpy as np

import concourse.bass as bass
import concourse.bacc as bacc
import concourse.tile as tile
import concourse.mybir as mybir
from concourse.bass_utils import run_bass_kernel_spmd

F32 = mybir.dt.float32
F16 = mybir.dt.float16
F8 = mybir.dt.float8e4

N = 16384          # nodes
D = 64             # feature dim (== H1 == H2 == H3)
G = 128            # graphs
NCLASS = 10
NCORES = 8
ROWS = N // NCORES          # 2048 output rows per core
KSUB = 4                    # 128-deep k-subtiles per mega-chunk
JC = 128 * KSUB             # contraction rows per mega-chunk (512)
NJC = N // JC               # 32 mega-chunks
NBANK = 4                   # psum accumulator banks per propagation
BCOLS = ROWS // NBANK       # 512 columns per bank
IB = ROWS // 128            # 16 output-row blocks of 128
NCACHE = 19                 # mega-chunks kept resident in SBUF
ASTREAM_BUFS = 4            # stream pool depth (covers the all-gather window)
LEAD = 4                    # cached chunks emitted before first streamed one

ADJ_DT_NAME = "float8e4"

_BUILD_CACHE = {}

# Timing-probe knob: replace the collectives with local DRAM copies
# (numerically wrong on 8 cores, but preserves the DMA/PE schedule).
NO_COLL = False

DR = mybir.MatmulPerfMode.DoubleRow


def _adj_dt():
    return F8


def _np_adj_dt():
    return mybir.dt.np(F8)


def _prop2_order():
    """Interleave cached mega-chunks among the streamed ones so the PE frees
    stream buffers at the pace DMA delivers them, with a small cached lead
    to cover the all-gather latency."""
    cached = list(range(NCACHE))
    streamed = list(range(NCACHE, NJC))
    order = cached[:LEAD]
    rem = cached[LEAD:]
    taken = 0
    for k, u in enumerate(streamed):
        order.append(u)
        want = (k + 1) * len(rem) // len(streamed)
        while taken < want:
            order.append(rem[taken])
            taken += 1
    order.extend(rem[taken:])
    assert sorted(order) == list(range(NJC))
    return order


def _build(adj_dt, reps=1):
    nc = bacc.Bacc("TRN2", target_bir_lowering=False, debug=False,
                   enable_asserts=True, num_devices=NCORES)

    at = nc.dram_tensor("at", [NJC * 128, KSUB * ROWS], F8,
                        kind="ExternalInput").ap()
    xw = nc.dram_tensor("xw", [128, N // 128 * D], F8,
                        kind="ExternalInput").ap()
    st = nc.dram_tensor("st", [128, IB * G], F16, kind="ExternalInput").ap()
    w1t = nc.dram_tensor("w1t", [D + 1, D], F16, kind="ExternalInput").ap()
    w2t = nc.dram_tensor("w2t", [D + 1, D], F16, kind="ExternalInput").ap()
    w3t = nc.dram_tensor("w3t", [D, D], F32, kind="ExternalInput").ap()
    b3 = nc.dram_tensor("b3", [D, 1], F32, kind="ExternalInput").ap()
    w4t = nc.dram_tensor("w4t", [D, NCLASS], F32, kind="ExternalInput").ap()
    b4 = nc.dram_tensor("b4", [NCLASS, 1], F32, kind="ExternalInput").ap()
    eye = nc.dram_tensor("eye", [32, 32], F32, kind="ExternalInput").ap()
    out = nc.dram_tensor("out", [G, NCLASS], F32, kind="ExternalOutput").ap()

    groups = [list(range(NCORES))]
    order2 = _prop2_order()

    with tile.TileContext(nc) as tc:
        with (
            tc.tile_pool(name="const", bufs=1) as const,
            tc.tile_pool(name="astream", bufs=ASTREAM_BUFS) as astream,
            tc.tile_pool(name="hp_pool", bufs=1) as hp_pool,
            tc.tile_pool(name="h1p", bufs=1) as h1p,
            tc.tile_pool(name="hb_pool", bufs=4) as hb_pool,
            tc.tile_pool(name="small", bufs=1) as small,
            tc.tile_pool(name="acc_pool", bufs=4, space="PSUM") as acc_pool,
            tc.tile_pool(name="pf_pool", bufs=3, space="PSUM") as pf_pool,
            tc.tile_pool(name="seg_pool", bufs=1, space="PSUM") as seg_pool,
            tc.tile_pool(name="dram", bufs=1, space="DRAM") as dram,
        ):
            # shared weight tile: x chunks during prop1, gathered h1 chunks
            # during prop2; layout [p, c128*128 + k*64 + d] (c128 = 128-row
            # chunk index, k = DoubleRow k-tile)
            wtile = const.tile([128, N // 128 * D], F8)
            w1t_s = const.tile([D + 1, D], F16)
            nc.scalar.dma_start(w1t_s[:], w1t[:])
            # adjacency cache, filled by prop1's mega-chunk DMAs
            acache = const.tile([128, KSUB * NCACHE, ROWS], F8)
            # constants only needed after prop1 (DMAs emitted later)
            st_all = const.tile([128, IB * G], F16)
            w2t_s = const.tile([D + 1, D], F16)
            w3t_s = const.tile([D, D], F32)
            b3_s = const.tile([D, 1], F32)
            w4t_s = const.tile([D, NCLASS], F32)
            b4_s = const.tile([NCLASS, 1], F32)
            eye_s = const.tile([32, 32], F32)

            def at_src(c):
                return at[c * 128:(c + 1) * 128, :].rearrange(
                    "p (k i) -> p k i", k=KSUB)

            def wslice(c128):
                return wtile[:, c128 * 2 * D:(c128 + 1) * 2 * D].rearrange(
                    "p (k d) -> p k d", k=2)

            def propagate(order, get_a):
                """One propagation: h.T accumulated over the mega-chunks
                (2 DoubleRow matmuls each) into 4 PSUM banks, then copied to
                a [65, 2048] fp16 SBUF tile with a trailing ones row (bias
                folding). Returns the SBUF tile."""
                acc = [
                    acc_pool.tile([D, BCOLS], F32, name=f"acc{b}", tag="acc")
                    for b in range(NBANK)
                ]
                first, last = order[0], order[-1]
                for c in order:
                    a4 = get_a(c)
                    for j in range(2):
                        w3d = wslice(c * 2 + j)
                        for b in range(NBANK):
                            nc.tensor.matmul(
                                acc[b][:],
                                w3d,
                                a4[:, 2 * j:2 * j + 2,
                                   b * BCOLS:(b + 1) * BCOLS],
                                start=(c == first and j == 0),
                                stop=(c == last and j == 1),
                                perf_mode=DR,
                            )
                hp = hp_pool.tile([D + 1, ROWS], F16, name="hp", tag="hp")
                nc.vector.memset(hp[D:D + 1, :], 1.0)
                for b in range(NBANK):
                    nc.vector.tensor_copy(hp[0:D, b * BCOLS:(b + 1) * BCOLS],
                                          acc[b][:])
                return hp

            for _rep in range(reps):
                # h1 in SBUF-partition-major layout [128, ib*64 + d]
                # (row c4*256 + k*128 + p -> h1_loc[p, (2*c4+k)*64 + d]) so
                # the fc1 write, the all-gather, and the bulk reload into
                # wtile are all single contiguous DMAs.
                h1_loc = dram.tile([128, IB * D], F8, name="h1_loc")
                h1_full = dram.tile([NCORES * 128, IB * D], F8,
                                    name="h1_full", addr_space="Shared")
                seg_loc = dram.tile([D, G], F32, name="seg_loc")
                seg_full = dram.tile([D, G], F32, name="seg_full",
                                     addr_space="Shared")

                # x chunks into the shared weight tile (one bulk DMA)
                nc.scalar.dma_start(wtile[:], xw[:])

                # ---- propagation 1: fill the cache, stream the rest ----
                def p1_a(c):
                    if c < NCACHE:
                        v = acache[:, KSUB * c:KSUB * (c + 1), :]
                        nc.sync.dma_start(v, at_src(c))
                        return v
                    a_t = astream.tile([128, KSUB, ROWS], F8, name="a_t",
                                       tag="a")
                    nc.sync.dma_start(a_t[:], at_src(c))
                    return a_t[:]

                hp1 = propagate(list(range(NJC)), p1_a)

                # constants for the later stages (DMA after prop1's stream)
                if _rep == 0:
                    nc.scalar.dma_start(st_all[:], st[:])
                    nc.scalar.dma_start(w2t_s[:], w2t[:])
                    nc.scalar.dma_start(w3t_s[:], w3t[:])
                    nc.scalar.dma_start(b3_s[:], b3[:])
                    nc.scalar.dma_start(w4t_s[:], w4t[:])
                    nc.scalar.dma_start(b4_s[:], b4[:])
                    nc.scalar.dma_start(eye_s[:], eye[:])

                # ---- fc1 (+bias, 1/16 fold) -> relu -> fp8 h1 -> DRAM ----
                # fc1 output lands in one SBUF tile so h1 ships as a single
                # DMA (one trigger on the pre-gather critical path, not 16).
                h1sb = h1p.tile([128, IB * D], F8, name="h1sb", tag="h1sb")
                for ib in range(IB):
                    pf = pf_pool.tile([128, D], F32, name="pf1", tag="pf")
                    nc.tensor.matmul(pf[:], hp1[:, ib * 128:(ib + 1) * 128],
                                     w1t_s[:], start=True, stop=True)
                    nc.scalar.activation(h1sb[:, ib * D:(ib + 1) * D], pf[:],
                                         mybir.ActivationFunctionType.Relu)
                nc.scalar.dma_start(h1_loc[:], h1sb[:])

                if NO_COLL:
                    nc.scalar.dma_start(h1_full[0:128, :], h1_loc[:])
                else:
                    nc.gpsimd.collective_compute(
                        "AllGather", mybir.AluOpType.bypass,
                        replica_groups=groups,
                        ins=[h1_loc.opt()], outs=[h1_full.opt()],
                    )

                # gathered h1 -> shared weight tile; one DMA per source core
                # block so prop2's first matmuls (low chunk ids) unblock as
                # soon as block 0 lands, not after the full megabyte.
                for co in range(NCORES):
                    nc.scalar.dma_start(
                        wtile[:, co * IB * D:(co + 1) * IB * D],
                        h1_full[co * 128:(co + 1) * 128, :])

                # ---- propagation 2: cached chunks free, stream the rest ----
                def p2_a(c):
                    if c < NCACHE:
                        return acache[:, KSUB * c:KSUB * (c + 1), :]
                    a_t = astream.tile([128, KSUB, ROWS], F8, name="a_t",
                                       tag="a")
                    nc.sync.dma_start(a_t[:], at_src(c))
                    return a_t[:]

                hp2 = propagate(order2, p2_a)

                # ---- fc2 (1/16 fold) -> relu -> fp16; readout via one-hot ----
                seg_ps = seg_pool.tile([D, G], F32, name="seg_ps", tag="seg")
                for ib in range(IB):
                    pf = pf_pool.tile([128, D], F32, name="pf2", tag="pf")
                    nc.tensor.matmul(pf[:], hp2[:, ib * 128:(ib + 1) * 128],
                                     w2t_s[:], start=True, stop=True)
                    hb2 = hb_pool.tile([128, D], F16, name="hb2", tag="hb2")
                    nc.scalar.activation(hb2[:], pf[:],
                                         mybir.ActivationFunctionType.Relu)
                    nc.tensor.matmul(seg_ps[:], hb2[:],
                                     st_all[:, ib * G:(ib + 1) * G],
                                     start=(ib == 0), stop=(ib == IB - 1))

                seg_s = small.tile([D, G], F32, name="seg_s", tag="seg_s")
                nc.vector.tensor_copy(seg_s[:], seg_ps[:])
                nc.scalar.dma_start(seg_loc[:], seg_s[:])
                if NO_COLL:
                    nc.scalar.dma_start(seg_full[:], seg_loc[:])
                else:
                    nc.gpsimd.collective_compute(
                        "AllReduce", mybir.AluOpType.add,
                        replica_groups=groups,
                        ins=[seg_loc.opt()], outs=[seg_full.opt()],
                    )
                segf_s = small.tile([D, G], F32, name="segf_s", tag="segf_s")
                nc.scalar.dma_start(segf_s[:], seg_full[:])

                # ---- readout MLP: fc3 relu, fc4 (+bias), all in .T layout ----
                p3 = pf_pool.tile([D, G], F32, name="p3", tag="pf")
                nc.tensor.matmul(p3[:], w3t_s[:], segf_s[:], start=True,
                                 stop=True)
                r3 = small.tile([D, G], F32, name="r3", tag="r3")
                nc.scalar.activation(r3[:], p3[:],
                                     mybir.ActivationFunctionType.Relu,
                                     bias=b3_s[:])
                p4 = pf_pool.tile([NCLASS, G], F32, name="p4", tag="pf")
                nc.tensor.matmul(p4[:], w4t_s[:], r3[:], start=True, stop=True)
                l4 = small.tile([NCLASS, G], F32, name="l4", tag="l4")
                nc.scalar.activation(l4[:], p4[:],
                                     mybir.ActivationFunctionType.Identity,
                                     bias=b4_s[:])

                # ---- transpose logits to [G, NCLASS]; log_softmax over free ----
                pt = pf_pool.tile([G, NCLASS], F32, name="pt", tag="pf")
                nc.tensor.transpose(pt[:], l4[:], eye_s[0:NCLASS, 0:NCLASS])
                negmx = small.tile([G, 1], F32, name="negmx", tag="negmx")
                nc.vector.tensor_reduce(negmx[:], pt[:],
                                        axis=mybir.AxisListType.X,
                                        op=mybir.AluOpType.max, negate=True)
                ex = small.tile([G, NCLASS], F32, name="ex", tag="ex")
                nc.scalar.activation(ex[:], pt[:],
                                     mybir.ActivationFunctionType.Exp,
                                     bias=negmx[:])
                sm = small.tile([G, 1], F32, name="sm", tag="sm")
                nc.vector.reduce_sum(sm[:], ex[:], axis=mybir.AxisListType.X)
                ls = small.tile([G, 1], F32, name="ls", tag="ls")
                nc.scalar.activation(ls[:], sm[:],
                                     mybir.ActivationFunctionType.Ln)
                res = small.tile([G, NCLASS], F32, name="res", tag="res")
                nc.vector.tensor_scalar(res[:], pt[:], negmx[:], ls[:],
                                        op0=mybir.AluOpType.add,
                                        op1=mybir.AluOpType.subtract)
                nc.scalar.dma_start(out[:], res[:])

    nc.compile()
    return nc


def _prep_inputs(inputs):
    """Host-side sharding/layout prep. Returns per-core input maps."""
    np_f8 = _np_adj_dt()
    x_in = np.ascontiguousarray(np.asarray(inputs["x_in"], dtype=np.float32))
    adj = np.asarray(inputs["adj"], dtype=np.float32)
    idx = np.asarray(inputs["idx"]).astype(np.int64)
    W1 = np.asarray(inputs["W1"], dtype=np.float32)
    b1 = np.asarray(inputs["b1"], dtype=np.float32)
    W2 = np.asarray(inputs["W2"], dtype=np.float32)
    b2 = np.asarray(inputs["b2"], dtype=np.float32)
    W3 = np.asarray(inputs["W3"], dtype=np.float32)
    b3 = np.asarray(inputs["b3"], dtype=np.float32)
    W4 = np.asarray(inputs["W4"], dtype=np.float32)
    b4 = np.asarray(inputs["b4"], dtype=np.float32)

    # x in chunked SBUF layout: xw[p, c128*128 + k*64 + d] =
    # x[c128*256 + k*128 + p, d]
    xq = x_in.astype(np_f8)
    xw = np.ascontiguousarray(
        xq.reshape(N // 256, 2, 128, D).transpose(2, 0, 1, 3)
        .reshape(128, N // 128 * D))

    # fc1 scales h1 by 1/16: keeps the fp8 h1 cast and the fp16 hp2 tile
    # (x2/16, max ~5.1e4 < 65504) in range; b2 is scaled to match so
    # hb2 = h2/16, and W3 carries the 16x compensation.
    w1t_aug = np.ascontiguousarray(
        (1.0 / 16.0) * np.concatenate([W1.T, b1[None, :]], axis=0)
    ).astype(np.float16)
    w2t_aug = np.ascontiguousarray(
        np.concatenate([W2.T, b2[None, :] / 16.0], axis=0)
    ).astype(np.float16)
    w3t = np.ascontiguousarray(16.0 * W3.T)
    w4t = np.ascontiguousarray(W4.T)
    b3c = np.ascontiguousarray(b3.reshape(D, 1))
    b4c = np.ascontiguousarray(b4.reshape(NCLASS, 1))
    eye = np.eye(32, dtype=np.float32)

    shared = {
        "xw": xw, "w1t": w1t_aug, "w2t": w2t_aug, "w3t": w3t,
        "b3": b3c, "w4t": w4t, "b4": b4c, "eye": eye,
    }

    in_maps = []
    for c in range(NCORES):
        r0 = c * ROWS
        at_c = np.ascontiguousarray(adj[r0:r0 + ROWS, :].T)  # [N, ROWS] f32
        at_c[r0 + np.arange(ROWS), np.arange(ROWS)] += 1.0   # self-loops
        at8 = at_c.astype(np_f8)
        # mega-chunk layout: at[(c, p), (k, i)] = A_T[c*512 + k*128 + p, i]
        at8 = np.ascontiguousarray(
            at8.reshape(NJC, KSUB, 128, ROWS).transpose(0, 2, 1, 3)
            .reshape(NJC * 128, KSUB * ROWS))

        # one-hot segment matrix in SBUF layout: st[p, b*G + g] =
        # 1 if idx[r0 + b*128 + p] == g
        st_c = np.zeros((128, IB * G), dtype=np.float16)
        loc = idx[r0:r0 + ROWS]
        p = np.arange(ROWS) % 128
        blk = np.arange(ROWS) // 128
        st_c[p, blk * G + loc] = 1.0

        in_maps.append({"at": at8, "st": st_c, **shared})
    return in_maps


def run(inputs, trace=False):
    """Build (cached), shard, execute on 8 cores; returns (out, results)."""
    key = ("k", 1)
    if key not in _BUILD_CACHE:
        _BUILD_CACHE[key] = _build(F8, reps=1)
    nc = _BUILD_CACHE[key]
    in_maps = _prep_inputs(inputs)
    res = run_bass_kernel_spmd(nc, in_maps, core_ids=list(range(NCORES)),
                               trace=trace)
    return np.asarray(res.results[0]["out"], dtype=np.float32), res


def _pjrt_timed_fn(nc, in_maps):
    """Build a persistent jitted runner over device-resident inputs.
    Returns a callable that executes the NEFF and returns wall seconds."""
    from concourse import bass2jax
    import jax

    bass2jax.install_neuronx_cc_hook()

    partition_name = (nc.partition_id_tensor.name
                      if nc.partition_id_tensor else None)
    in_names, out_names, out_avals, zero_outs = [], [], [], []
    for alloc in nc.m.functions[0].allocations:
        if not isinstance(alloc, mybir.MemoryLocationSet):
            continue
        name = alloc.memorylocations[0].name
        if alloc.kind == "ExternalInput":
            if name != partition_name:
                in_names.append(name)
        elif alloc.kind == "ExternalOutput":
            shape = tuple(alloc.tensor_shape)
            dtype = mybir.dt.np(alloc.dtype)
            out_names.append(name)
            out_avals.append(jax.core.ShapedArray(shape, dtype))
            zero_outs.append(np.zeros(shape, dtype))
    n_params = len(in_names)
    n_outs = len(out_avals)
    all_names = in_names + out_names
    if partition_name is not None:
        all_names = all_names + [partition_name]
    donate = tuple(range(n_params, n_params + n_outs))

    def _body(*args):
        operands = list(args)
        if partition_name is not None:
            operands.append(bass2jax.partition_id_tensor())
        outs = bass2jax._bass_exec_p.bind(
            *operands,
            out_avals=tuple(out_avals),
            in_names=tuple(all_names),
            out_names=tuple(out_names),
            lowering_input_output_aliases=(),
            sim_require_finite=True,
            sim_require_nnan=True,
            nc=nc,
        )
        return tuple(outs)

    devices = jax.devices()[:NCORES]
    mesh = bass2jax.Mesh(np.asarray(devices), ("core",))
    P = bass2jax.PartitionSpec
    in_specs = (P("core"),) * (n_params + n_outs)
    out_specs = (P("core"),) * n_outs
    sharded = jax.jit(
        bass2jax.shard_map(_body, mesh=mesh, in_specs=in_specs,
                           out_specs=out_specs, check_rep=False),
        donate_argnums=donate, keep_unused=True)

    concat_in = [
        np.concatenate([np.asarray(in_maps[c][nm]) for c in range(NCORES)],
                       axis=0)
        for nm in in_names
    ]
    from jax.sharding import NamedSharding
    dev_in = [jax.device_put(a, NamedSharding(mesh, P("core")))
              for a in concat_in]

    def call_once():
        zeros = [np.zeros((NCORES * z.shape[0], *z.shape[1:]), z.dtype)
                 for z in zero_outs]
        t0 = time.perf_counter()
        outs = sharded(*dev_in, *zeros)
        jax.block_until_ready(outs)
        return time.perf_counter() - t0

    return call_once


def measure_hw_ns(inputs, reps_lo=1, reps_hi=33, iters=40):
    """Estimate per-invocation HW exec time via a reps-delta: the kernel
    body replicated R-x inside one NEFF (executions pipeline across rep
    boundaries, so this is steady-state throughput). Per-call dispatch
    overhead is ~4ms with multi-ms bimodal drift, so the two arms are
    called adjacently in time and the median of per-pair differences is
    used (robust to mode flips)."""
    in_maps = _prep_inputs(inputs)
    fns = {}
    for reps in (reps_lo, reps_hi):
        key = ("k", reps)
        if key not in _BUILD_CACHE:
            _BUILD_CACHE[key] = _build(F8, reps=reps)
        fns[reps] = _pjrt_timed_fn(_BUILD_CACHE[key], in_maps)
        fns[reps]()  # warmup (compile + first exec)
    diffs = []
    for _ in range(iters):
        ta = fns[reps_lo]()
        tb = fns[reps_hi]()
        diffs.append((tb - ta) / (reps_hi - reps_lo))
    est = float(np.median(diffs)) * 1e9
    stats = {"diffs_us": sorted(round(d * 1e6, 1) for d in diffs)}
    return est, stats


def kernel(**inputs):
    out, _ = run(inputs, trace=False)
    return out


# revision 33
# speedup vs baseline: 1.1249x; 1.1249x over previous
"""Trainium2 Bass kernel for a 2-layer GNN (dense message passing) + MLP readout.

Reference computation (N=16384 nodes, D=64 features, G=128 graphs):
    adj_hat = adj + I
    x  = adj_hat @ x_in                 # prop 1
    x  = relu(x @ W1.T + b1)            # fc1
    x  = adj_hat @ x                    # prop 2
    x  = relu(x @ W2.T + b2)            # fc2
    out = segment_sum(x, idx, 128)      # readout
    out = relu(out @ W3.T + b3)
    out = out @ W4.T + b4
    return log_softmax(out, axis=1)

Sharding over 8 NeuronCores: row-shard adj_hat over output nodes (2048 rows
per core), pre-transposed on the host with self-loops folded in so the
contraction index j lands on SBUF partitions.

Performance design (per core, DMA-roofline bound):
 - adjacency/x/h1 in fp8e4 + MatmulPerfMode.DoubleRow: halves HBM bytes vs
   fp16 and runs the PE at full 128-wide rate (the 64-wide feature dim
   would otherwise idle half the array).
 - the adjacency shard streams as 32 mega-chunks of [512 j, 2048 i]
   (1 MiB each, 8 KiB per partition per DMA); 19 of them stay resident in
   SBUF after prop 1 so prop 2 re-reads only 13 from HBM.
 - DMA instruction count is kept minimal (~55/rep): every dma_start
   serializes ~0.65us on the global HWDGE descriptor generator, so x and
   the gathered h1 are loaded with one bulk DMA each into a shared weight
   tile, and h1 is written back with a single DMA.
 - h1 is scaled by 1/16 (folded into W1, compensated in W2/W3) so its fp8
   cast and the fp16 hp tiles stay in range.
"""

import os
import sys
import time

for _p in ("/opt/trn_rl_repo",):
    if _p not in sys.path and os.path.isdir(_p):
        sys.path.insert(0, _p)

import numpy as np

import concourse.bass as bass
import concourse.bacc as bacc
import concourse.tile as tile
import concourse.mybir as mybir
from concourse.bass_utils import run_bass_kernel_spmd

F32 = mybir.dt.float32
F16 = mybir.dt.float16
F8 = mybir.dt.float8e4

N = 16384          # nodes
D = 64             # feature dim (== H1 == H2 == H3)
G = 128            # graphs
NCLASS = 10
NCORES = 8
ROWS = N // NCORES          # 2048 output rows per core
KSUB = 4                    # 128-deep k-subtiles per mega-chunk
JC = 128 * KSUB             # contraction rows per mega-chunk (512)
NJC = N // JC               # 32 mega-chunks
NBANK = 4                   # psum accumulator banks per propagation
BCOLS = ROWS // NBANK       # 512 columns per bank
IB = ROWS // 128            # 16 output-row blocks of 128
NCACHE = 20                 # mega-chunks kept resident in SBUF
ASTREAM_BUFS = 3            # stream pool depth (covers the all-gather window)
LEAD = 4                    # cached chunks emitted before first streamed one

ADJ_DT_NAME = "float8e4"

_BUILD_CACHE = {}

# Timing-probe knob: replace the collectives with local DRAM copies
# (numerically wrong on 8 cores, but preserves the DMA/PE schedule).
NO_COLL = False

DR = mybir.MatmulPerfMode.DoubleRow


def _adj_dt():
    return F8


def _np_adj_dt():
    return mybir.dt.np(F8)


def _prop2_order():
    """Interleave cached mega-chunks among the streamed ones so the PE frees
    stream buffers at the pace DMA delivers them, with a small cached lead
    to cover the all-gather latency."""
    cached = list(range(NCACHE))
    streamed = list(range(NCACHE, NJC))
    order = cached[:LEAD]
    rem = cached[LEAD:]
    taken = 0
    for k, u in enumerate(streamed):
        order.append(u)
        want = (k + 1) * len(rem) // len(streamed)
        while taken < want:
            order.append(rem[taken])
            taken += 1
    order.extend(rem[taken:])
    assert sorted(order) == list(range(NJC))
    return order


def _build(adj_dt, reps=1):
    nc = bacc.Bacc("TRN2", target_bir_lowering=False, debug=False,
                   enable_asserts=True, num_devices=NCORES)

    at = nc.dram_tensor("at", [NJC * 128, KSUB * ROWS], F8,
                        kind="ExternalInput").ap()
    xw = nc.dram_tensor("xw", [128, N // 128 * D], F8,
                        kind="ExternalInput").ap()
    st = nc.dram_tensor("st", [128, IB * G], F16, kind="ExternalInput").ap()
    w1t = nc.dram_tensor("w1t", [D + 1, D], F16, kind="ExternalInput").ap()
    w2t = nc.dram_tensor("w2t", [D + 1, D], F16, kind="ExternalInput").ap()
    w3t = nc.dram_tensor("w3t", [D, D], F32, kind="ExternalInput").ap()
    b3 = nc.dram_tensor("b3", [D, 1], F32, kind="ExternalInput").ap()
    w4t = nc.dram_tensor("w4t", [D, NCLASS], F32, kind="ExternalInput").ap()
    b4 = nc.dram_tensor("b4", [NCLASS, 1], F32, kind="ExternalInput").ap()
    eye = nc.dram_tensor("eye", [32, 32], F32, kind="ExternalInput").ap()
    out = nc.dram_tensor("out", [G, NCLASS], F32, kind="ExternalOutput").ap()

    groups = [list(range(NCORES))]
    order2 = _prop2_order()

    with tile.TileContext(nc) as tc:
        with (
            tc.tile_pool(name="const", bufs=1) as const,
            tc.tile_pool(name="astream", bufs=ASTREAM_BUFS) as astream,
            tc.tile_pool(name="hp_pool", bufs=1) as hp_pool,
            tc.tile_pool(name="h1p", bufs=1) as h1p,
            tc.tile_pool(name="hb_pool", bufs=4) as hb_pool,
            tc.tile_pool(name="small", bufs=1) as small,
            tc.tile_pool(name="acc_pool", bufs=4, space="PSUM") as acc_pool,
            tc.tile_pool(name="pf_pool", bufs=3, space="PSUM") as pf_pool,
            tc.tile_pool(name="seg_pool", bufs=1, space="PSUM") as seg_pool,
            tc.tile_pool(name="dram", bufs=1, space="DRAM") as dram,
        ):
            # shared weight tile: x chunks during prop1, gathered h1 chunks
            # during prop2; layout [p, c128*128 + k*64 + d] (c128 = 128-row
            # chunk index, k = DoubleRow k-tile)
            wtile = const.tile([128, N // 128 * D], F8)
            w1t_s = const.tile([D + 1, D], F16)
            nc.scalar.dma_start(w1t_s[:], w1t[:])
            # adjacency cache, filled by prop1's mega-chunk DMAs
            acache = const.tile([128, KSUB * NCACHE, ROWS], F8)
            # constants only needed after prop1 (DMAs emitted later)
            st_all = const.tile([128, IB * G], F16)
            w2t_s = const.tile([D + 1, D], F16)
            w3t_s = const.tile([D, D], F32)
            b3_s = const.tile([D, 1], F32)
            w4t_s = const.tile([D, NCLASS], F32)
            b4_s = const.tile([NCLASS, 1], F32)
            eye_s = const.tile([32, 32], F32)

            def at_src(c):
                return at[c * 128:(c + 1) * 128, :].rearrange(
                    "p (k i) -> p k i", k=KSUB)

            def wslice(c128):
                return wtile[:, c128 * 2 * D:(c128 + 1) * 2 * D].rearrange(
                    "p (k d) -> p k d", k=2)

            def propagate(order, get_a):
                """One propagation: h.T accumulated over the mega-chunks
                (2 DoubleRow matmuls each) into 4 PSUM banks, then copied to
                a [65, 2048] fp16 SBUF tile with a trailing ones row (bias
                folding). Returns the SBUF tile."""
                acc = [
                    acc_pool.tile([D, BCOLS], F32, name=f"acc{b}", tag="acc")
                    for b in range(NBANK)
                ]
                first, last = order[0], order[-1]
                for c in order:
                    a4 = get_a(c)
                    for j in range(2):
                        w3d = wslice(c * 2 + j)
                        for b in range(NBANK):
                            nc.tensor.matmul(
                                acc[b][:],
                                w3d,
                                a4[:, 2 * j:2 * j + 2,
                                   b * BCOLS:(b + 1) * BCOLS],
                                start=(c == first and j == 0),
                                stop=(c == last and j == 1),
                                perf_mode=DR,
                            )
                hp = hp_pool.tile([D + 1, ROWS], F16, name="hp", tag="hp")
                nc.vector.memset(hp[D:D + 1, :], 1.0)
                for b in range(NBANK):
                    nc.vector.tensor_copy(hp[0:D, b * BCOLS:(b + 1) * BCOLS],
                                          acc[b][:])
                return hp

            for _rep in range(reps):
                # h1 in SBUF-partition-major layout [128, ib*64 + d]
                # (row c4*256 + k*128 + p -> h1_loc[p, (2*c4+k)*64 + d]) so
                # the fc1 write, the all-gather, and the bulk reload into
                # wtile are all single contiguous DMAs.
                h1_loc = dram.tile([128, IB * D], F8, name="h1_loc")
                h1_full = dram.tile([NCORES * 128, IB * D], F8,
                                    name="h1_full", addr_space="Shared")
                seg_loc = dram.tile([D, G], F32, name="seg_loc")
                seg_full = dram.tile([D, G], F32, name="seg_full",
                                     addr_space="Shared")

                # x chunks into the shared weight tile (one bulk DMA)
                nc.scalar.dma_start(wtile[:], xw[:])

                # ---- propagation 1: fill the cache, stream the rest ----
                def p1_a(c):
                    if c < NCACHE:
                        v = acache[:, KSUB * c:KSUB * (c + 1), :]
                        nc.sync.dma_start(v, at_src(c))
                        return v
                    a_t = astream.tile([128, KSUB, ROWS], F8, name="a_t",
                                       tag="a")
                    nc.sync.dma_start(a_t[:], at_src(c))
                    return a_t[:]

                hp1 = propagate(list(range(NJC)), p1_a)

                # constants for the later stages (DMA after prop1's stream)
                if _rep == 0:
                    nc.scalar.dma_start(st_all[:], st[:])
                    nc.scalar.dma_start(w2t_s[:], w2t[:])
                    nc.scalar.dma_start(w3t_s[:], w3t[:])
                    nc.scalar.dma_start(b3_s[:], b3[:])
                    nc.scalar.dma_start(w4t_s[:], w4t[:])
                    nc.scalar.dma_start(b4_s[:], b4[:])
                    nc.scalar.dma_start(eye_s[:], eye[:])

                # ---- fc1 (+bias, 1/16 fold) -> relu -> fp8 h1 -> DRAM ----
                # fc1 output lands in one SBUF tile so h1 ships as a single
                # DMA (one trigger on the pre-gather critical path, not 16).
                h1sb = h1p.tile([128, IB * D], F8, name="h1sb", tag="h1sb")
                for ib in range(IB):
                    pf = pf_pool.tile([128, D], F32, name="pf1", tag="pf")
                    nc.tensor.matmul(pf[:], hp1[:, ib * 128:(ib + 1) * 128],
                                     w1t_s[:], start=True, stop=True)
                    nc.scalar.activation(h1sb[:, ib * D:(ib + 1) * D], pf[:],
                                         mybir.ActivationFunctionType.Relu)
                nc.scalar.dma_start(h1_loc[:], h1sb[:])

                if NO_COLL:
                    nc.scalar.dma_start(h1_full[0:128, :], h1_loc[:])
                else:
                    nc.gpsimd.collective_compute(
                        "AllGather", mybir.AluOpType.bypass,
                        replica_groups=groups,
                        ins=[h1_loc.opt()], outs=[h1_full.opt()],
                    )

                # gathered h1 -> shared weight tile; one DMA per source core
                # block so prop2's first matmuls (low chunk ids) unblock as
                # soon as block 0 lands, not after the full megabyte.
                for co in range(NCORES):
                    nc.scalar.dma_start(
                        wtile[:, co * IB * D:(co + 1) * IB * D],
                        h1_full[co * 128:(co + 1) * 128, :])

                # ---- propagation 2: cached chunks free; stream the rest,
                # first into the small stream pool, then into cache slots
                # whose chunks prop2 has already consumed (WAR deps keep
                # this safe) so the stream pool can stay tiny.
                def p2_a(c):
                    if c < NCACHE:
                        return acache[:, KSUB * c:KSUB * (c + 1), :]
                    i = c - NCACHE
                    if i < ASTREAM_BUFS:
                        a_t = astream.tile([128, KSUB, ROWS], F8, name="a_t",
                                           tag="a")
                        nc.sync.dma_start(a_t[:], at_src(c))
                        return a_t[:]
                    slot = i - ASTREAM_BUFS
                    v = acache[:, KSUB * slot:KSUB * (slot + 1), :]
                    nc.sync.dma_start(v, at_src(c))
                    return v

                hp2 = propagate(order2, p2_a)

                # ---- fc2 (1/16 fold) -> relu -> fp16; readout via one-hot ----
                seg_ps = seg_pool.tile([D, G], F32, name="seg_ps", tag="seg")
                for ib in range(IB):
                    pf = pf_pool.tile([128, D], F32, name="pf2", tag="pf")
                    nc.tensor.matmul(pf[:], hp2[:, ib * 128:(ib + 1) * 128],
                                     w2t_s[:], start=True, stop=True)
                    hb2 = hb_pool.tile([128, D], F16, name="hb2", tag="hb2")
                    nc.scalar.activation(hb2[:], pf[:],
                                         mybir.ActivationFunctionType.Relu)
                    nc.tensor.matmul(seg_ps[:], hb2[:],
                                     st_all[:, ib * G:(ib + 1) * G],
                                     start=(ib == 0), stop=(ib == IB - 1))

                seg_s = small.tile([D, G], F32, name="seg_s", tag="seg_s")
                nc.vector.tensor_copy(seg_s[:], seg_ps[:])
                nc.scalar.dma_start(seg_loc[:], seg_s[:])
                if NO_COLL:
                    nc.scalar.dma_start(seg_full[:], seg_loc[:])
                else:
                    nc.gpsimd.collective_compute(
                        "AllReduce", mybir.AluOpType.add,
                        replica_groups=groups,
                        ins=[seg_loc.opt()], outs=[seg_full.opt()],
                    )
                segf_s = small.tile([D, G], F32, name="segf_s", tag="segf_s")
                nc.scalar.dma_start(segf_s[:], seg_full[:])

                # ---- readout MLP: fc3 relu, fc4 (+bias), all in .T layout ----
                p3 = pf_pool.tile([D, G], F32, name="p3", tag="pf")
                nc.tensor.matmul(p3[:], w3t_s[:], segf_s[:], start=True,
                                 stop=True)
                r3 = small.tile([D, G], F32, name="r3", tag="r3")
                nc.scalar.activation(r3[:], p3[:],
                                     mybir.ActivationFunctionType.Relu,
                                     bias=b3_s[:])
                p4 = pf_pool.tile([NCLASS, G], F32, name="p4", tag="pf")
                nc.tensor.matmul(p4[:], w4t_s[:], r3[:], start=True, stop=True)
                l4 = small.tile([NCLASS, G], F32, name="l4", tag="l4")
                nc.scalar.activation(l4[:], p4[:],
                                     mybir.ActivationFunctionType.Identity,
                                     bias=b4_s[:])

                # ---- transpose logits to [G, NCLASS]; log_softmax over free ----
                pt = pf_pool.tile([G, NCLASS], F32, name="pt", tag="pf")
                nc.tensor.transpose(pt[:], l4[:], eye_s[0:NCLASS, 0:NCLASS])
                negmx = small.tile([G, 1], F32, name="negmx", tag="negmx")
                nc.vector.tensor_reduce(negmx[:], pt[:],
                                        axis=mybir.AxisListType.X,
                                        op=mybir.AluOpType.max, negate=True)
                ex = small.tile([G, NCLASS], F32, name="ex", tag="ex")
                nc.scalar.activation(ex[:], pt[:],
                                     mybir.ActivationFunctionType.Exp,
                                     bias=negmx[:])
                sm = small.tile([G, 1], F32, name="sm", tag="sm")
                nc.vector.reduce_sum(sm[:], ex[:], axis=mybir.AxisListType.X)
                ls = small.tile([G, 1], F32, name="ls", tag="ls")
                nc.scalar.activation(ls[:], sm[:],
                                     mybir.ActivationFunctionType.Ln)
                res = small.tile([G, NCLASS], F32, name="res", tag="res")
                nc.vector.tensor_scalar(res[:], pt[:], negmx[:], ls[:],
                                        op0=mybir.AluOpType.add,
                                        op1=mybir.AluOpType.subtract)
                nc.scalar.dma_start(out[:], res[:])

    nc.compile()
    return nc


def _prep_inputs(inputs):
    """Host-side sharding/layout prep. Returns per-core input maps."""
    np_f8 = _np_adj_dt()
    x_in = np.ascontiguousarray(np.asarray(inputs["x_in"], dtype=np.float32))
    adj = np.asarray(inputs["adj"], dtype=np.float32)
    idx = np.asarray(inputs["idx"]).astype(np.int64)
    W1 = np.asarray(inputs["W1"], dtype=np.float32)
    b1 = np.asarray(inputs["b1"], dtype=np.float32)
    W2 = np.asarray(inputs["W2"], dtype=np.float32)
    b2 = np.asarray(inputs["b2"], dtype=np.float32)
    W3 = np.asarray(inputs["W3"], dtype=np.float32)
    b3 = np.asarray(inputs["b3"], dtype=np.float32)
    W4 = np.asarray(inputs["W4"], dtype=np.float32)
    b4 = np.asarray(inputs["b4"], dtype=np.float32)

    # x in chunked SBUF layout: xw[p, c128*128 + k*64 + d] =
    # x[c128*256 + k*128 + p, d]
    xq = x_in.astype(np_f8)
    xw = np.ascontiguousarray(
        xq.reshape(N // 256, 2, 128, D).transpose(2, 0, 1, 3)
        .reshape(128, N // 128 * D))

    # fc1 scales h1 by 1/16: keeps the fp8 h1 cast and the fp16 hp2 tile
    # (x2/16, max ~5.1e4 < 65504) in range; b2 is scaled to match so
    # hb2 = h2/16, and W3 carries the 16x compensation.
    w1t_aug = np.ascontiguousarray(
        (1.0 / 16.0) * np.concatenate([W1.T, b1[None, :]], axis=0)
    ).astype(np.float16)
    w2t_aug = np.ascontiguousarray(
        np.concatenate([W2.T, b2[None, :] / 16.0], axis=0)
    ).astype(np.float16)
    w3t = np.ascontiguousarray(16.0 * W3.T)
    w4t = np.ascontiguousarray(W4.T)
    b3c = np.ascontiguousarray(b3.reshape(D, 1))
    b4c = np.ascontiguousarray(b4.reshape(NCLASS, 1))
    eye = np.eye(32, dtype=np.float32)

    shared = {
        "xw": xw, "w1t": w1t_aug, "w2t": w2t_aug, "w3t": w3t,
        "b3": b3c, "w4t": w4t, "b4": b4c, "eye": eye,
    }

    in_maps = []
    for c in range(NCORES):
        r0 = c * ROWS
        at_c = np.ascontiguousarray(adj[r0:r0 + ROWS, :].T)  # [N, ROWS] f32
        at_c[r0 + np.arange(ROWS), np.arange(ROWS)] += 1.0   # self-loops
        at8 = at_c.astype(np_f8)
        # mega-chunk layout: at[(c, p), (k, i)] = A_T[c*512 + k*128 + p, i]
        at8 = np.ascontiguousarray(
            at8.reshape(NJC, KSUB, 128, ROWS).transpose(0, 2, 1, 3)
            .reshape(NJC * 128, KSUB * ROWS))

        # one-hot segment matrix in SBUF layout: st[p, b*G + g] =
        # 1 if idx[r0 + b*128 + p] == g
        st_c = np.zeros((128, IB * G), dtype=np.float16)
        loc = idx[r0:r0 + ROWS]
        p = np.arange(ROWS) % 128
        blk = np.arange(ROWS) // 128
        st_c[p, blk * G + loc] = 1.0

        in_maps.append({"at": at8, "st": st_c, **shared})
    return in_maps


def run(inputs, trace=False):
    """Build (cached), shard, execute on 8 cores; returns (out, results)."""
    key = ("k", 1)
    if key not in _BUILD_CACHE:
        _BUILD_CACHE[key] = _build(F8, reps=1)
    nc = _BUILD_CACHE[key]
    in_maps = _prep_inputs(inputs)
    res = run_bass_kernel_spmd(nc, in_maps, core_ids=list(range(NCORES)),
                               trace=trace)
    return np.asarray(res.results[0]["out"], dtype=np.float32), res


def _pjrt_timed_fn(nc, in_maps):
    """Build a persistent jitted runner over device-resident inputs.
    Returns a callable that executes the NEFF and returns wall seconds."""
    from concourse import bass2jax
    import jax

    bass2jax.install_neuronx_cc_hook()

    partition_name = (nc.partition_id_tensor.name
                      if nc.partition_id_tensor else None)
    in_names, out_names, out_avals, zero_outs = [], [], [], []
    for alloc in nc.m.functions[0].allocations:
        if not isinstance(alloc, mybir.MemoryLocationSet):
            continue
        name = alloc.memorylocations[0].name
        if alloc.kind == "ExternalInput":
            if name != partition_name:
                in_names.append(name)
        elif alloc.kind == "ExternalOutput":
            shape = tuple(alloc.tensor_shape)
            dtype = mybir.dt.np(alloc.dtype)
            out_names.append(name)
            out_avals.append(jax.core.ShapedArray(shape, dtype))
            zero_outs.append(np.zeros(shape, dtype))
    n_params = len(in_names)
    n_outs = len(out_avals)
    all_names = in_names + out_names
    if partition_name is not None:
        all_names = all_names + [partition_name]
    donate = tuple(range(n_params, n_params + n_outs))

    def _body(*args):
        operands = list(args)
        if partition_name is not None:
            operands.append(bass2jax.partition_id_tensor())
        outs = bass2jax._bass_exec_p.bind(
            *operands,
            out_avals=tuple(out_avals),
            in_names=tuple(all_names),
            out_names=tuple(out_names),
            lowering_input_output_aliases=(),
            sim_require_finite=True,
            sim_require_nnan=True,
            nc=nc,
        )
        return tuple(outs)

    devices = jax.devices()[:NCORES]
    mesh = bass2jax.Mesh(np.asarray(devices), ("core",))
    P = bass2jax.PartitionSpec
    in_specs = (P("core"),) * (n_params + n_outs)
    out_specs = (P("core"),) * n_outs
    sharded = jax.jit(
        bass2jax.shard_map(_body, mesh=mesh, in_specs=in_specs,
                           out_specs=out_specs, check_rep=False),
        donate_argnums=donate, keep_unused=True)

    concat_in = [
        np.concatenate([np.asarray(in_maps[c][nm]) for c in range(NCORES)],
                       axis=0)
        for nm in in_names
    ]
    from jax.sharding import NamedSharding
    dev_in = [jax.device_put(a, NamedSharding(mesh, P("core")))
              for a in concat_in]

    def call_once():
        zeros = [np.zeros((NCORES * z.shape[0], *z.shape[1:]), z.dtype)
                 for z in zero_outs]
        t0 = time.perf_counter()
        outs = sharded(*dev_in, *zeros)
        jax.block_until_ready(outs)
        return time.perf_counter() - t0

    return call_once


def measure_hw_ns(inputs, reps_lo=1, reps_hi=33, iters=40):
    """Estimate per-invocation HW exec time via a reps-delta: the kernel
    body replicated R-x inside one NEFF (executions pipeline across rep
    boundaries, so this is steady-state throughput). Per-call dispatch
    overhead is ~4ms with multi-ms bimodal drift, so the two arms are
    called adjacently in time and the median of per-pair differences is
    used (robust to mode flips)."""
    in_maps = _prep_inputs(inputs)
    fns = {}
    for reps in (reps_lo, reps_hi):
        key = ("k", reps)
        if key not in _BUILD_CACHE:
            _BUILD_CACHE[key] = _build(F8, reps=reps)
        fns[reps] = _pjrt_timed_fn(_BUILD_CACHE[key], in_maps)
        fns[reps]()  # warmup (compile + first exec)
    diffs = []
    for _ in range(iters):
        ta = fns[reps_lo]()
        tb = fns[reps_hi]()
        diffs.append((tb - ta) / (reps_hi - reps_lo))
    est = float(np.median(diffs)) * 1e9
    stats = {"diffs_us": sorted(round(d * 1e6, 1) for d in diffs)}
    return est, stats


def kernel(**inputs):
    out, _ = run(inputs, trace=False)
    return out
